# revision 33
# baseline (speedup 1.0000x reference)
"""Banded causal self-attention (B=1, T=4096, C=1024, H=16, Dh=64, band=128)
on 8 Trainium2 NeuronCores, sequence-parallel (512 queries/core + 128-row halo).

v3 layout strategy (vs baseline):
  - additive band masks are preloaded into the score PSUM banks by the PE
    itself (identity-stationary matmul, start=True), so the score matmuls
    accumulate on top and the DVE mask-add disappears; exp+accum reads PSUM
    directly.
  - P normalization is one broadcast multiply per (g,h) on GpSimd (idle
    engine) instead of 4 tensor_scalars on DVE.
  - attention front work (scores/exp/normalize/transpose) for group g is
    interleaved with the V projection in program order so the scalar/DVE/Pool
    engines run under the projection matmuls; PV runs once v lands.
  - output projection is split into two half-contractions so the first half
    overlaps the attention tail.
"""

import os
import sys

import ml_dtypes
import numpy as np

sys.path.insert(0, "/opt/trn_rl_repo")

import concourse.bass as bass  # noqa: F401
import concourse.mybir as mybir
import concourse.tile as tile
from concourse import bacc
from concourse.bass_utils import run_bass_kernel_spmd
from concourse.masks import make_identity

T, C, H, DH = 4096, 1024, 16, 64
BAND = 128
NCORES = 8
TLOC = T // NCORES          # 512 queries per core
HALO = BAND                 # 128
KV = TLOC + HALO            # 640 kv rows per core
NQB = TLOC // 128           # 4 query blocks
NKB = KV // 128             # 5 kv blocks
KT = C // 128               # 8 contraction tiles
F32 = mybir.dt.float32
BF16 = mybir.dt.bfloat16
MULT = mybir.AluOpType.mult
ADD = mybir.AluOpType.add
EXP = mybir.ActivationFunctionType.Exp

NORM_MODE = os.environ.get("KERNEL_NORM_MODE", "vector_ts")

_cached = {}


def build_nc():
    nc = bacc.Bacc(
        "TRN2",
        target_bir_lowering=False,
        debug=False,
        num_devices=NCORES,
    )

    xt_d = nc.dram_tensor("xt", [C, KV], BF16, kind="ExternalInput")
    wqt_d = nc.dram_tensor("wqt", [C, C], BF16, kind="ExternalInput")
    wkt_d = nc.dram_tensor("wkt", [C, C], BF16, kind="ExternalInput")
    wvt_d = nc.dram_tensor("wvt", [C, C], BF16, kind="ExternalInput")
    wot_d = nc.dram_tensor("wot", [C, C], BF16, kind="ExternalInput")
    # additive band masks per [q 128, 512] score tile: madd0 covers q-blocks
    # (0,1) (core-variant: core 0's halo half fully masked), maddr covers the
    # generic window pair.
    madd0_d = nc.dram_tensor("madd0", [128, 512], BF16, kind="ExternalInput")
    maddr_d = nc.dram_tensor("maddr", [128, 512], BF16, kind="ExternalInput")
    out_d = nc.dram_tensor("out", [TLOC, C], F32, kind="ExternalOutput")

    with tile.TileContext(nc) as tc:
        with (
            tc.tile_pool(name="const", bufs=1) as constp,
            tc.tile_pool(name="xt", bufs=KT) as xtp,
            tc.tile_pool(name="w", bufs=16) as wp,
            tc.tile_pool(name="qt", bufs=KT) as qtp,
            tc.tile_pool(name="kt", bufs=KT) as ktp,
            tc.tile_pool(name="v", bufs=NKB) as vp,
            tc.tile_pool(name="yt", bufs=KT) as ytp,
            tc.tile_pool(name="att", bufs=6) as attp,
            tc.tile_pool(name="pt", bufs=16) as ptp,
            tc.tile_pool(name="stat", bufs=4) as statp,
            tc.tile_pool(name="z", bufs=4) as zp,
            tc.tile_pool(name="psum", bufs=1, space="PSUM") as psp,
        ):
            # HAM warm-up: junk matmuls that run while the first DMAs land,
            # flipping the PE clock gate to 8/8 before real work begins
            junk = constp.tile([128, 512], BF16, name="junk")
            nc.vector.memset(junk[:], 0.0)
            ps_w = psp.tile([128, 512], F32, tag="y", bufs=2, name="warm")
            for _ in range(2):
                nc.tensor.matmul(ps_w[:], junk[:, 0:128], junk[:], start=True,
                                 stop=True)
            ident = constp.tile([128, 128], BF16, name="ident")
            make_identity(nc, ident[:])

            madd0 = constp.tile([128, 512], BF16, name="madd0")
            maddr = constp.tile([128, 512], BF16, name="maddr")
            nc.sync.dma_start(madd0[:], madd0_d[:])
            nc.sync.dma_start(maddr[:], maddr_d[:])

            def load_w(dram, base, k):
                w = wp.tile([128, C], BF16, name=f"{base}{k}", tag="w", bufs=16)
                nc.sync.dma_start(w[:], dram[k * 128:(k + 1) * 128, :])
                return w

            # interleave x^T and Wq tile loads so the first q-projection
            # accumulation chain starts as early as possible
            xt_t, wq_t = [], []
            for a in range(KT):
                xt = xtp.tile([128, KV], BF16, name=f"xt{a}", tag="xt", bufs=KT)
                nc.sync.dma_start(xt[:], xt_d[a * 128:(a + 1) * 128, :])
                xt_t.append(xt)
                wq_t.append(load_w(wqt_d, "wq", a))
            wk_t = [load_w(wkt_d, "wk", k) for k in range(KT)]

            # ---- q^T projection: out (o, t) tiles [128, 512]
            qt_t = []
            for o in range(KT):
                ps = psp.tile([128, 512], F32, tag="proj", bufs=3, name=f"psq{o}")
                for k in range(KT):
                    nc.tensor.matmul(
                        ps[:],
                        wq_t[k][:, o * 128:(o + 1) * 128],
                        xt_t[k][:, HALO:],
                        start=(k == 0),
                        stop=(k == KT - 1),
                    )
                qt = qtp.tile([128, TLOC], BF16, name=f"qt{o}", tag="qt", bufs=KT)
                nc.vector.tensor_copy(qt[:], ps[:])
                qt_t.append(qt)

            # ---- k^T projection: out (o, t) tiles [128, 640]
            kt_t = []
            for o in range(KT):
                kt = ktp.tile([128, KV], BF16, name=f"kt{o}", tag="kt", bufs=KT)
                for n0, nw in ((0, 384), (384, 256)):
                    ps = psp.tile([128, 512], F32, tag="proj", bufs=3,
                                  name=f"psk{o}_{n0}")
                    for k in range(KT):
                        nc.tensor.matmul(
                            ps[:, :nw],
                            wk_t[k][:, o * 128:(o + 1) * 128],
                            xt_t[k][:, n0:n0 + nw],
                            start=(k == 0),
                            stop=(k == KT - 1),
                        )
                    nc.vector.tensor_copy(kt[:, n0:n0 + nw], ps[:, :nw])
                kt_t.append(kt)

            wv_t = [load_w(wvt_d, "wv", k) for k in range(KT)]
            wo_t = [load_w(wot_d, "wo", k) for k in range(KT)]

            # ---- attention front for head-pair g: scores + softmax + P^T.
            # Only needs qt[g]/kt[g]; interleaved with the V projection.
            pt_all = [None] * KT

            p_all = [None] * KT

            def attn_front(g):
                den = statp.tile([128, 8], F32, tag="den", bufs=4,
                                 name=f"den{g}")
                rec = statp.tile([128, 8], F32, tag="rec", bufs=4,
                                 name=f"rec{g}")
                e_t = {}
                for h in (0, 1):
                    ho = h * 64
                    e = attp.tile([128, 4 * 256], BF16, tag="e", bufs=6,
                                  name=f"e{g}_{h}")
                    e_t[h] = e
                    for qp in range(2):
                        madd = madd0 if qp == 0 else maddr
                        ps_s = psp.tile([128, 512], F32, tag="s", bufs=3,
                                        name=f"s{g}_{h}_{qp}")
                        for i in range(2):
                            qb = 2 * qp + i
                            # PE preloads the additive mask into PSUM, then
                            # the scores accumulate on top
                            nc.tensor.matmul(
                                ps_s[:, i * 256:(i + 1) * 256],
                                ident[:], madd[:, i * 256:(i + 1) * 256],
                                start=True, stop=False,
                                skip_group_check=True)
                            nc.tensor.matmul(
                                ps_s[:, i * 256:(i + 1) * 256],
                                qt_t[g][ho:ho + 64, qb * 128:(qb + 1) * 128],
                                kt_t[g][ho:ho + 64, qb * 128:qb * 128 + 256],
                                start=False,
                                stop=True,
                                skip_group_check=True,
                            )
                        # exp straight out of PSUM with per-window row sums
                        for i in range(2):
                            qb = 2 * qp + i
                            nc.scalar.activation(
                                e[:, qb * 256:(qb + 1) * 256],
                                ps_s[:, i * 256:(i + 1) * 256], EXP,
                                accum_out=den[:, h * 4 + qb:h * 4 + qb + 1])
                nc.vector.reciprocal(rec[:], den[:])
                for h in (0, 1):
                    p = attp.tile([128, 4 * 256], BF16, tag="p", bufs=6,
                                  name=f"p{g}_{h}")
                    p_all[g] = p_all[g] or {}
                    p_all[g][h] = p
                    if NORM_MODE == "gpsimd_bcast":
                        nc.gpsimd.tensor_tensor(
                            out=p[:].rearrange("p (b w) -> p b w", b=4),
                            in0=e_t[h][:].rearrange("p (b w) -> p b w", b=4),
                            in1=rec[:, h * 4:(h + 1) * 4].unsqueeze(-1)
                                .broadcast_to([128, 4, 256]),
                            op=MULT,
                        )
                    else:
                        for qb in range(NQB):
                            nc.vector.tensor_scalar_mul(
                                p[:, qb * 256:(qb + 1) * 256],
                                e_t[h][:, qb * 256:(qb + 1) * 256],
                                rec[:, h * 4 + qb:h * 4 + qb + 1])
            def attn_tposes(g):
                # P^T via PE transposes; pt window layout: kv block jb's
                # 256 q-cols are [128(jb-1), 128(jb+1)).
                for h in (0, 1):
                    p = p_all[g][h]
                    pt = ptp.tile([128, 256 * NKB], BF16, tag="pt", bufs=16,
                                  name=f"pt{g}_{h}")
                    for qp in range(2):
                        ps_t = psp.tile([128, 512], BF16, tag="s", bufs=3,
                                        name=f"t{g}_{h}_{qp}")
                        for i in range(2):
                            qb = 2 * qp + i
                            nc.tensor.transpose(
                                ps_t[:, i * 256:i * 256 + 128],
                                p[:, qb * 256:qb * 256 + 128], ident[:])
                            nc.tensor.transpose(
                                ps_t[:, i * 256 + 128:i * 256 + 256],
                                p[:, qb * 256 + 128:qb * 256 + 256],
                                ident[:])
                        for i in range(2):
                            qb = 2 * qp + i
                            nc.vector.tensor_copy(
                                pt[:, qb * 256 + 128:qb * 256 + 256],
                                ps_t[:, i * 256:i * 256 + 128])
                            nc.vector.tensor_copy(
                                pt[:, (qb + 1) * 256:(qb + 1) * 256 + 128],
                                ps_t[:, i * 256 + 128:i * 256 + 256])
                    pt_all[g] = pt_all[g] or {}
                    pt_all[g][h] = pt

            def v_chain(tb):
                v = vp.tile([128, C], BF16, name=f"v{tb}", tag="v", bufs=NKB)
                for n0 in (0, 512):
                    ps = psp.tile([128, 512], F32, tag="proj", bufs=3,
                                  name=f"psv{tb}_{n0}")
                    for k in range(KT):
                        nc.tensor.matmul(
                            ps[:],
                            xt_t[k][:, tb * 128:(tb + 1) * 128],
                            wv_t[k][:, n0:n0 + 512],
                            start=(k == 0),
                            stop=(k == KT - 1),
                        )
                    nc.vector.tensor_copy(v[:, n0:n0 + 512], ps[:])
                return v


            # ---- PV per head pair
            yt_t = [None] * KT

            def attn_pv(g):
                # start-flag-split accumulation: each q-block region of one
                # PSUM tile is its own 2-instruction group (kv blocks jb=b
                # then jb=b+1), so a single bank holds the full y and the
                # next group pipelines into the other buffer
                y = psp.tile([128, TLOC], F32, tag="y", bufs=2, name=f"y{g}")
                for jb in range(NKB):
                    for h in (0, 1):
                        ho = h * 64
                        for b in (jb - 1, jb):
                            if not (0 <= b < NQB):
                                continue
                            pc = jb * 256 + (0 if b == jb - 1 else 128)
                            nc.tensor.matmul(
                                y[ho:ho + 64, b * 128:(b + 1) * 128],
                                v_t[jb][:, (2 * g + h) * 64:
                                        (2 * g + h + 1) * 64],
                                pt_all[g][h][:, pc:pc + 128],
                                start=(jb == b),
                                stop=(jb == b + 1),
                                tile_position=(0, ho) if ho else None,
                                skip_group_check=True,
                            )
                yt = ytp.tile([128, TLOC], BF16, name=f"yt{g}", tag="yt",
                              bufs=KT)
                nc.scalar.copy(yt[:], y[:])
                yt_t[g] = yt

            # interleave: softmax fronts run one group ahead of their
            # transposes so the PE never blocks on the DVE normalize; V
            # chains and PV groups weave between
            v_t = []
            attn_front(0)
            v_t.append(v_chain(0))
            for g in range(1, NKB):
                attn_front(g)
                attn_tposes(g - 1)
                v_t.append(v_chain(g))
            attn_front(5)
            attn_tposes(4)
            attn_pv(0)
            attn_front(6)
            attn_tposes(5)
            attn_pv(1)
            attn_front(7)
            attn_tposes(6)
            attn_pv(2)
            attn_tposes(7)
            attn_pv(3)
            attn_pv(4)
            attn_pv(5)

            # ---- output projection z = y @ Wo^T, split into halves so the
            # first half overlaps the attention tail
            za_t = []

            def z_half_a():
                for tb in range(NQB):
                    for n0 in (0, 512):
                        ps = psp.tile([128, 512], F32, tag="proj", bufs=3,
                                      name=f"psza{tb}_{n0}")
                        for o in range(6):
                            nc.tensor.matmul(
                                ps[:],
                                yt_t[o][:, tb * 128:(tb + 1) * 128],
                                wo_t[o][:, n0:n0 + 512],
                                start=(o == 0),
                                stop=(o == 5),
                            )
                        za = zp.tile([128, 512], F32, name=f"za{tb}_{n0}",
                                     tag="za", bufs=8)
                        nc.scalar.copy(za[:], ps[:])
                        za_t.append(za)

            z_half_a()
            for g in range(6, KT):
                attn_pv(g)

            for tb in range(NQB):
                zt = zp.tile([128, C], F32, name=f"z{tb}", tag="z", bufs=2)
                for ni, n0 in enumerate((0, 512)):
                    ps = psp.tile([128, 512], F32, tag="proj", bufs=3,
                                  name=f"pszb{tb}_{n0}")
                    for o in range(6, KT):
                        nc.tensor.matmul(
                            ps[:],
                            yt_t[o][:, tb * 128:(tb + 1) * 128],
                            wo_t[o][:, n0:n0 + 512],
                            start=(o == 6),
                            stop=(o == KT - 1),
                        )
                    nc.vector.tensor_tensor(out=zt[:, n0:n0 + 512], in0=ps[:],
                                            in1=za_t[tb * 2 + ni][:], op=ADD)
                eng = nc.sync if tb % 2 == 0 else nc.scalar
                eng.dma_start(out_d[tb * 128:(tb + 1) * 128, :], zt[:])

    nc.compile()
    return nc


def _masks():
    # additive masks for a [q 128, 512] score pair-tile; each 256-wide
    # window: cols 0:128 = prev kv block (valid iff k > r, strictly upper),
    # cols 128:256 = diag kv block (valid iff k-128 <= r).
    r = np.arange(128)[:, None]
    k = np.arange(256)[None, :]
    prev = (k < 128) & (k > r)
    diag = (k >= 128) & ((k - 128) <= r)
    win = np.where(prev | diag, 0.0, -1e9).astype(np.float32)
    first0 = np.where(diag & ~prev, 0.0, -1e9).astype(np.float32)
    maddr = np.concatenate([win, win], axis=1)
    madd0 = np.concatenate([win, win], axis=1)
    madd0_c0 = np.concatenate([first0, win], axis=1)
    return (madd0.astype(ml_dtypes.bfloat16),
            madd0_c0.astype(ml_dtypes.bfloat16),
            maddr.astype(ml_dtypes.bfloat16))


def make_in_maps(x, Wq, Wk, Wv, Wo):
    x = np.asarray(x, dtype=np.float32)
    xt = np.ascontiguousarray(x.reshape(T, C).T.astype(ml_dtypes.bfloat16))
    wqt = np.ascontiguousarray(
        (np.asarray(Wq, np.float32).T * np.float32(1.0 / np.sqrt(DH))
         ).astype(ml_dtypes.bfloat16))
    wkt = np.ascontiguousarray(np.asarray(Wk, np.float32).T.astype(ml_dtypes.bfloat16))
    wvt = np.ascontiguousarray(np.asarray(Wv, np.float32).T.astype(ml_dtypes.bfloat16))
    wot = np.ascontiguousarray(np.asarray(Wo, np.float32).T.astype(ml_dtypes.bfloat16))
    madd0, madd0_c0, maddr = _masks()

    in_maps = []
    for c in range(NCORES):
        t0 = c * TLOC
        xs = np.zeros((C, KV), dtype=ml_dtypes.bfloat16)
        lo = t0 - HALO
        src_lo = max(lo, 0)
        xs[:, src_lo - lo:] = xt[:, src_lo:t0 + TLOC]
        in_maps.append(
            {
                "xt": xs,
                "wqt": wqt,
                "wkt": wkt,
                "wvt": wvt,
                "wot": wot,
                "madd0": madd0_c0 if c == 0 else madd0,
                "maddr": maddr,
            }
        )
    return in_maps


def get_nc():
    if "nc" not in _cached:
        _cached["nc"] = build_nc()
    return _cached["nc"]


def kernel(x, Wq, Wk, Wv, Wo):
    in_maps = make_in_maps(x, Wq, Wk, Wv, Wo)
    res = run_bass_kernel_spmd(get_nc(), in_maps, list(range(NCORES)))
    out = np.concatenate([res.results[c]["out"] for c in range(NCORES)], axis=0)
    return out.reshape(1, T, C)


if __name__ == "__main__":
    rng = np.random.default_rng(0)
    ins = {
        "x": rng.standard_normal((1, T, C), dtype=np.float32),
        "Wq": rng.standard_normal((C, C), dtype=np.float32) * 0.02,
        "Wk": rng.standard_normal((C, C), dtype=np.float32) * 0.02,
        "Wv": rng.standard_normal((C, C), dtype=np.float32) * 0.02,
        "Wo": rng.standard_normal((C, C), dtype=np.float32) * 0.02,
    }
    out = kernel(**ins)
    print(out.shape, out.dtype, np.abs(out).mean())


# revision 34
# speedup vs baseline: 1.1896x; 1.1896x over previous
"""Banded causal self-attention (B=1, T=4096, C=1024, H=16, Dh=64, band=128)
on 8 Trainium2 NeuronCores, sequence-parallel (512 queries/core + 128-row halo).

v3 layout strategy (vs baseline):
  - additive band masks are preloaded into the score PSUM banks by the PE
    itself (identity-stationary matmul, start=True), so the score matmuls
    accumulate on top and the DVE mask-add disappears; exp+accum reads PSUM
    directly.
  - P normalization is one broadcast multiply per (g,h) on GpSimd (idle
    engine) instead of 4 tensor_scalars on DVE.
  - attention front work (scores/exp/normalize/transpose) for group g is
    interleaved with the V projection in program order so the scalar/DVE/Pool
    engines run under the projection matmuls; PV runs once v lands.
  - output projection is split into two half-contractions so the first half
    overlaps the attention tail.
"""

import os
import sys

import ml_dtypes
import numpy as np

sys.path.insert(0, "/opt/trn_rl_repo")

import concourse.bass as bass  # noqa: F401
import concourse.mybir as mybir
import concourse.tile as tile
from concourse import bacc
from concourse.bass_utils import run_bass_kernel_spmd
from concourse.masks import make_identity

T, C, H, DH = 4096, 1024, 16, 64
BAND = 128
NCORES = 8
TLOC = T // NCORES          # 512 queries per core
HALO = BAND                 # 128
KV = TLOC + HALO            # 640 kv rows per core
NQB = TLOC // 128           # 4 query blocks
NKB = KV // 128             # 5 kv blocks
KT = C // 128               # 8 contraction tiles
F32 = mybir.dt.float32
BF16 = mybir.dt.bfloat16
MULT = mybir.AluOpType.mult
ADD = mybir.AluOpType.add
EXP = mybir.ActivationFunctionType.Exp

NORM_MODE = os.environ.get("KERNEL_NORM_MODE", "vector_ts")

_cached = {}


def build_nc():
    nc = bacc.Bacc(
        "TRN2",
        target_bir_lowering=False,
        debug=False,
        num_devices=NCORES,
    )

    xt_d = nc.dram_tensor("xt", [C, KV], BF16, kind="ExternalInput")
    wqt_d = nc.dram_tensor("wqt", [C, C], BF16, kind="ExternalInput")
    wkt_d = nc.dram_tensor("wkt", [C, C], BF16, kind="ExternalInput")
    wvt_d = nc.dram_tensor("wvt", [C, C], BF16, kind="ExternalInput")
    wot_d = nc.dram_tensor("wot", [C, C], BF16, kind="ExternalInput")
    # additive band masks per [q 128, 512] score tile: madd0 covers q-blocks
    # (0,1) (core-variant: core 0's halo half fully masked), maddr covers the
    # generic window pair.
    madd0_d = nc.dram_tensor("madd0", [128, 512], BF16, kind="ExternalInput")
    maddr_d = nc.dram_tensor("maddr", [128, 512], BF16, kind="ExternalInput")
    out_d = nc.dram_tensor("out", [TLOC, C], F32, kind="ExternalOutput")

    with tile.TileContext(nc) as tc:
        with (
            tc.tile_pool(name="const", bufs=1) as constp,
            tc.tile_pool(name="xt", bufs=KT) as xtp,
            tc.tile_pool(name="w", bufs=16) as wp,
            tc.tile_pool(name="qt", bufs=KT) as qtp,
            tc.tile_pool(name="kt", bufs=KT) as ktp,
            tc.tile_pool(name="v", bufs=NKB) as vp,
            tc.tile_pool(name="yt", bufs=KT) as ytp,
            tc.tile_pool(name="att", bufs=6) as attp,
            tc.tile_pool(name="pt", bufs=16) as ptp,
            tc.tile_pool(name="stat", bufs=4) as statp,
            tc.tile_pool(name="z", bufs=4) as zp,
            tc.tile_pool(name="psum", bufs=1, space="PSUM") as psp,
        ):
            # HAM warm-up: junk matmuls that run while the first DMAs land,
            # flipping the PE clock gate to 8/8 before real work begins
            junk = constp.tile([128, 512], BF16, name="junk")
            nc.vector.memset(junk[:], 0.0)
            ps_w = psp.tile([128, 512], F32, tag="y", bufs=2, name="warm")
            for _ in range(4):
                nc.tensor.matmul(ps_w[:], junk[:, 0:128], junk[:], start=True,
                                 stop=True)
            ident = constp.tile([128, 128], BF16, name="ident")
            make_identity(nc, ident[:])

            madd0 = constp.tile([128, 512], BF16, name="madd0")
            maddr = constp.tile([128, 512], BF16, name="maddr")
            nc.sync.dma_start(madd0[:], madd0_d[:])
            nc.sync.dma_start(maddr[:], maddr_d[:])

            def load_w(dram, base, k):
                w = wp.tile([128, C], BF16, name=f"{base}{k}", tag="w", bufs=16)
                nc.sync.dma_start(w[:], dram[k * 128:(k + 1) * 128, :])
                return w

            # interleave x^T and Wq tile loads so the first q-projection
            # accumulation chain starts as early as possible
            xt_t, wq_t = [], []
            for a in range(KT):
                xt = xtp.tile([128, KV], BF16, name=f"xt{a}", tag="xt", bufs=KT)
                nc.sync.dma_start(xt[:], xt_d[a * 128:(a + 1) * 128, :])
                xt_t.append(xt)
                wq_t.append(load_w(wqt_d, "wq", a))
            wk_t = [load_w(wkt_d, "wk", k) for k in range(KT)]

            # ---- q^T projection: out (o, t) tiles [128, 512]
            qt_t = []
            for o in range(KT):
                ps = psp.tile([128, 512], F32, tag="proj", bufs=3, name=f"psq{o}")
                for k in range(KT):
                    nc.tensor.matmul(
                        ps[:],
                        wq_t[k][:, o * 128:(o + 1) * 128],
                        xt_t[k][:, HALO:],
                        start=(k == 0),
                        stop=(k == KT - 1),
                    )
                qt = qtp.tile([128, TLOC], BF16, name=f"qt{o}", tag="qt", bufs=KT)
                nc.vector.tensor_copy(qt[:], ps[:])
                qt_t.append(qt)

            # ---- k^T projection: out (o, t) tiles [128, 640]
            kt_t = []
            for o in range(KT):
                kt = ktp.tile([128, KV], BF16, name=f"kt{o}", tag="kt", bufs=KT)
                for n0, nw in ((0, 384), (384, 256)):
                    ps = psp.tile([128, 512], F32, tag="proj", bufs=3,
                                  name=f"psk{o}_{n0}")
                    for k in range(KT):
                        nc.tensor.matmul(
                            ps[:, :nw],
                            wk_t[k][:, o * 128:(o + 1) * 128],
                            xt_t[k][:, n0:n0 + nw],
                            start=(k == 0),
                            stop=(k == KT - 1),
                        )
                    nc.vector.tensor_copy(kt[:, n0:n0 + nw], ps[:, :nw])
                kt_t.append(kt)

            wv_t = [load_w(wvt_d, "wv", k) for k in range(KT)]
            wo_t = [load_w(wot_d, "wo", k) for k in range(KT)]

            # ---- attention front for head-pair g: scores + softmax + P^T.
            # Only needs qt[g]/kt[g]; interleaved with the V projection.
            pt_all = [None] * KT

            p_all = [None] * KT

            def attn_front(g):
                den = statp.tile([128, 8], F32, tag="den", bufs=4,
                                 name=f"den{g}")
                rec = statp.tile([128, 8], F32, tag="rec", bufs=4,
                                 name=f"rec{g}")
                e_t = {}
                for h in (0, 1):
                    ho = h * 64
                    e = attp.tile([128, 4 * 256], BF16, tag="e", bufs=6,
                                  name=f"e{g}_{h}")
                    e_t[h] = e
                    for qp in range(2):
                        madd = madd0 if qp == 0 else maddr
                        ps_s = psp.tile([128, 512], F32, tag="s", bufs=3,
                                        name=f"s{g}_{h}_{qp}")
                        for i in range(2):
                            qb = 2 * qp + i
                            # PE preloads the additive mask into PSUM, then
                            # the scores accumulate on top
                            nc.tensor.matmul(
                                ps_s[:, i * 256:(i + 1) * 256],
                                ident[:], madd[:, i * 256:(i + 1) * 256],
                                start=True, stop=False,
                                skip_group_check=True)
                            nc.tensor.matmul(
                                ps_s[:, i * 256:(i + 1) * 256],
                                qt_t[g][ho:ho + 64, qb * 128:(qb + 1) * 128],
                                kt_t[g][ho:ho + 64, qb * 128:qb * 128 + 256],
                                start=False,
                                stop=True,
                                skip_group_check=True,
                            )
                        # exp straight out of PSUM with per-window row sums
                        for i in range(2):
                            qb = 2 * qp + i
                            nc.scalar.activation(
                                e[:, qb * 256:(qb + 1) * 256],
                                ps_s[:, i * 256:(i + 1) * 256], EXP,
                                accum_out=den[:, h * 4 + qb:h * 4 + qb + 1])
                nc.vector.reciprocal(rec[:], den[:])
                for h in (0, 1):
                    p = attp.tile([128, 4 * 256], BF16, tag="p", bufs=6,
                                  name=f"p{g}_{h}")
                    p_all[g] = p_all[g] or {}
                    p_all[g][h] = p
                    if NORM_MODE == "gpsimd_bcast":
                        nc.gpsimd.tensor_tensor(
                            out=p[:].rearrange("p (b w) -> p b w", b=4),
                            in0=e_t[h][:].rearrange("p (b w) -> p b w", b=4),
                            in1=rec[:, h * 4:(h + 1) * 4].unsqueeze(-1)
                                .broadcast_to([128, 4, 256]),
                            op=MULT,
                        )
                    else:
                        for qb in range(NQB):
                            nc.vector.tensor_scalar_mul(
                                p[:, qb * 256:(qb + 1) * 256],
                                e_t[h][:, qb * 256:(qb + 1) * 256],
                                rec[:, h * 4 + qb:h * 4 + qb + 1])
            def attn_tposes(g):
                # P^T via PE transposes; pt window layout: kv block jb's
                # 256 q-cols are [128(jb-1), 128(jb+1)).
                for h in (0, 1):
                    p = p_all[g][h]
                    pt = ptp.tile([128, 256 * NKB], BF16, tag="pt", bufs=16,
                                  name=f"pt{g}_{h}")
                    for qp in range(2):
                        ps_t = psp.tile([128, 512], BF16, tag="s", bufs=3,
                                        name=f"t{g}_{h}_{qp}")
                        for i in range(2):
                            qb = 2 * qp + i
                            nc.tensor.transpose(
                                ps_t[:, i * 256:i * 256 + 128],
                                p[:, qb * 256:qb * 256 + 128], ident[:])
                            nc.tensor.transpose(
                                ps_t[:, i * 256 + 128:i * 256 + 256],
                                p[:, qb * 256 + 128:qb * 256 + 256],
                                ident[:])
                        for i in range(2):
                            qb = 2 * qp + i
                            nc.vector.tensor_copy(
                                pt[:, qb * 256 + 128:qb * 256 + 256],
                                ps_t[:, i * 256:i * 256 + 128])
                            nc.vector.tensor_copy(
                                pt[:, (qb + 1) * 256:(qb + 1) * 256 + 128],
                                ps_t[:, i * 256 + 128:i * 256 + 256])
                    pt_all[g] = pt_all[g] or {}
                    pt_all[g][h] = pt

            def v_chain(tb):
                v = vp.tile([128, C], BF16, name=f"v{tb}", tag="v", bufs=NKB)
                for n0 in (0, 512):
                    ps = psp.tile([128, 512], F32, tag="proj", bufs=3,
                                  name=f"psv{tb}_{n0}")
                    for k in range(KT):
                        nc.tensor.matmul(
                            ps[:],
                            xt_t[k][:, tb * 128:(tb + 1) * 128],
                            wv_t[k][:, n0:n0 + 512],
                            start=(k == 0),
                            stop=(k == KT - 1),
                        )
                    nc.vector.tensor_copy(v[:, n0:n0 + 512], ps[:])
                return v


            # ---- PV per head pair
            yt_t = [None] * KT

            def attn_pv(g):
                # start-flag-split accumulation: each q-block region of one
                # PSUM tile is its own 2-instruction group (kv blocks jb=b
                # then jb=b+1), so a single bank holds the full y and the
                # next group pipelines into the other buffer
                y = psp.tile([128, TLOC], F32, tag="y", bufs=2, name=f"y{g}")
                for jb in range(NKB):
                    for h in (0, 1):
                        ho = h * 64
                        for b in (jb - 1, jb):
                            if not (0 <= b < NQB):
                                continue
                            pc = jb * 256 + (0 if b == jb - 1 else 128)
                            nc.tensor.matmul(
                                y[ho:ho + 64, b * 128:(b + 1) * 128],
                                v_t[jb][:, (2 * g + h) * 64:
                                        (2 * g + h + 1) * 64],
                                pt_all[g][h][:, pc:pc + 128],
                                start=(jb == b),
                                stop=(jb == b + 1),
                                tile_position=(0, ho) if ho else None,
                                skip_group_check=True,
                            )
                yt = ytp.tile([128, TLOC], BF16, name=f"yt{g}", tag="yt",
                              bufs=KT)
                nc.scalar.copy(yt[:], y[:])
                yt_t[g] = yt

            # interleave: softmax fronts run one group ahead of their
            # transposes so the PE never blocks on the DVE normalize; V
            # chains and PV groups weave between
            v_t = []
            attn_front(0)
            v_t.append(v_chain(0))
            for g in range(1, NKB):
                attn_front(g)
                attn_tposes(g - 1)
                v_t.append(v_chain(g))
            attn_front(5)
            attn_tposes(4)
            attn_pv(0)
            attn_front(6)
            attn_tposes(5)
            attn_pv(1)
            attn_front(7)
            attn_tposes(6)
            attn_pv(2)
            attn_tposes(7)
            attn_pv(3)
            attn_pv(4)
            attn_pv(5)

            # ---- output projection z = y @ Wo^T, split into halves so the
            # first half overlaps the attention tail
            za_t = []

            def z_half_a():
                for tb in range(NQB):
                    for n0 in (0, 512):
                        ps = psp.tile([128, 512], F32, tag="proj", bufs=3,
                                      name=f"psza{tb}_{n0}")
                        for o in range(6):
                            nc.tensor.matmul(
                                ps[:],
                                yt_t[o][:, tb * 128:(tb + 1) * 128],
                                wo_t[o][:, n0:n0 + 512],
                                start=(o == 0),
                                stop=(o == 5),
                            )
                        za = zp.tile([128, 512], F32, name=f"za{tb}_{n0}",
                                     tag="za", bufs=8)
                        nc.vector.tensor_copy(za[:], ps[:])
                        za_t.append(za)

            z_half_a()
            for g in range(6, KT):
                attn_pv(g)

            for tb in range(NQB):
                zt = zp.tile([128, C], F32, name=f"z{tb}", tag="z", bufs=2)
                for ni, n0 in enumerate((0, 512)):
                    ps = psp.tile([128, 512], F32, tag="proj", bufs=3,
                                  name=f"pszb{tb}_{n0}")
                    for o in range(6, KT):
                        nc.tensor.matmul(
                            ps[:],
                            yt_t[o][:, tb * 128:(tb + 1) * 128],
                            wo_t[o][:, n0:n0 + 512],
                            start=(o == 6),
                            stop=(o == KT - 1),
                        )
                    nc.vector.tensor_tensor(out=zt[:, n0:n0 + 512], in0=ps[:],
                                            in1=za_t[tb * 2 + ni][:], op=ADD)
                eng = nc.sync if tb % 2 == 0 else nc.scalar
                eng.dma_start(out_d[tb * 128:(tb + 1) * 128, :], zt[:])

    nc.compile()
    return nc


def _masks():
    # additive masks for a [q 128, 512] score pair-tile; each 256-wide
    # window: cols 0:128 = prev kv block (valid iff k > r, strictly upper),
    # cols 128:256 = diag kv block (valid iff k-128 <= r).
    r = np.arange(128)[:, None]
    k = np.arange(256)[None, :]
    prev = (k < 128) & (k > r)
    diag = (k >= 128) & ((k - 128) <= r)
    win = np.where(prev | diag, 0.0, -1e9).astype(np.float32)
    first0 = np.where(diag & ~prev, 0.0, -1e9).astype(np.float32)
    maddr = np.concatenate([win, win], axis=1)
    madd0 = np.concatenate([win, win], axis=1)
    madd0_c0 = np.concatenate([first0, win], axis=1)
    return (madd0.astype(ml_dtypes.bfloat16),
            madd0_c0.astype(ml_dtypes.bfloat16),
            maddr.astype(ml_dtypes.bfloat16))


def make_in_maps(x, Wq, Wk, Wv, Wo):
    x = np.asarray(x, dtype=np.float32)
    xt = np.ascontiguousarray(x.reshape(T, C).T.astype(ml_dtypes.bfloat16))
    wqt = np.ascontiguousarray(
        (np.asarray(Wq, np.float32).T * np.float32(1.0 / np.sqrt(DH))
         ).astype(ml_dtypes.bfloat16))
    wkt = np.ascontiguousarray(np.asarray(Wk, np.float32).T.astype(ml_dtypes.bfloat16))
    wvt = np.ascontiguousarray(np.asarray(Wv, np.float32).T.astype(ml_dtypes.bfloat16))
    wot = np.ascontiguousarray(np.asarray(Wo, np.float32).T.astype(ml_dtypes.bfloat16))
    madd0, madd0_c0, maddr = _masks()

    in_maps = []
    for c in range(NCORES):
        t0 = c * TLOC
        xs = np.zeros((C, KV), dtype=ml_dtypes.bfloat16)
        lo = t0 - HALO
        src_lo = max(lo, 0)
        xs[:, src_lo - lo:] = xt[:, src_lo:t0 + TLOC]
        in_maps.append(
            {
                "xt": xs,
                "wqt": wqt,
                "wkt": wkt,
                "wvt": wvt,
                "wot": wot,
                "madd0": madd0_c0 if c == 0 else madd0,
                "maddr": maddr,
            }
        )
    return in_maps


def get_nc():
    if "nc" not in _cached:
        _cached["nc"] = build_nc()
    return _cached["nc"]


def kernel(x, Wq, Wk, Wv, Wo):
    in_maps = make_in_maps(x, Wq, Wk, Wv, Wo)
    res = run_bass_kernel_spmd(get_nc(), in_maps, list(range(NCORES)))
    out = np.concatenate([res.results[c]["out"] for c in range(NCORES)], axis=0)
    return out.reshape(1, T, C)


if __name__ == "__main__":
    rng = np.random.default_rng(0)
    ins = {
        "x": rng.standard_normal((1, T, C), dtype=np.float32),
        "Wq": rng.standard_normal((C, C), dtype=np.float32) * 0.02,
        "Wk": rng.standard_normal((C, C), dtype=np.float32) * 0.02,
        "Wv": rng.standard_normal((C, C), dtype=np.float32) * 0.02,
        "Wo": rng.standard_normal((C, C), dtype=np.float32) * 0.02,
    }
    out = kernel(**ins)
    print(out.shape, out.dtype, np.abs(out).mean())


# revision 35
# speedup vs baseline: 1.3338x; 1.1213x over previous
"""Banded causal self-attention (B=1, T=4096, C=1024, H=16, Dh=64, band=128)
on 8 Trainium2 NeuronCores, sequence-parallel (512 queries/core + 128-row halo).

v3 layout strategy (vs baseline):
  - additive band masks are preloaded into the score PSUM banks by the PE
    itself (identity-stationary matmul, start=True), so the score matmuls
    accumulate on top and the DVE mask-add disappears; exp+accum reads PSUM
    directly.
  - P normalization is one broadcast multiply per (g,h) on GpSimd (idle
    engine) instead of 4 tensor_scalars on DVE.
  - attention front work (scores/exp/normalize/transpose) for group g is
    interleaved with the V projection in program order so the scalar/DVE/Pool
    engines run under the projection matmuls; PV runs once v lands.
  - output projection is split into two half-contractions so the first half
    overlaps the attention tail.
"""

import os
import sys

import ml_dtypes
import numpy as np

sys.path.insert(0, "/opt/trn_rl_repo")

import concourse.bass as bass  # noqa: F401
import concourse.mybir as mybir
import concourse.tile as tile
from concourse import bacc
from concourse.bass_utils import run_bass_kernel_spmd
from concourse.masks import make_identity

T, C, H, DH = 4096, 1024, 16, 64
BAND = 128
NCORES = 8
TLOC = T // NCORES          # 512 queries per core
HALO = BAND                 # 128
KV = TLOC + HALO            # 640 kv rows per core
NQB = TLOC // 128           # 4 query blocks
NKB = KV // 128             # 5 kv blocks
KT = C // 128               # 8 contraction tiles
F32 = mybir.dt.float32
BF16 = mybir.dt.bfloat16
FP8 = mybir.dt.float8e4
MULT = mybir.AluOpType.mult
ADD = mybir.AluOpType.add
EXP = mybir.ActivationFunctionType.Exp

NORM_MODE = os.environ.get("KERNEL_NORM_MODE", "vector_ts")

_cached = {}


def build_nc():
    nc = bacc.Bacc(
        "TRN2",
        target_bir_lowering=False,
        debug=False,
        num_devices=NCORES,
    )

    xt_d = nc.dram_tensor("xt", [C, KV], BF16, kind="ExternalInput")
    # fp8 DoubleRow operands for the q/k projections: k-tile pairs
    # interleaved along the free dim ([128, 2, cols] per pair)
    xt8_d = nc.dram_tensor("xt8", [C // 2, 2 * KV], FP8, kind="ExternalInput")
    wqt8_d = nc.dram_tensor("wqt8", [C // 2, 2 * C], FP8, kind="ExternalInput")
    wkt8_d = nc.dram_tensor("wkt8", [C // 2, 2 * C], FP8, kind="ExternalInput")
    wvt_d = nc.dram_tensor("wvt", [C, C], BF16, kind="ExternalInput")
    wot_d = nc.dram_tensor("wot", [C, C], BF16, kind="ExternalInput")
    # additive band masks per [q 128, 512] score tile: madd0 covers q-blocks
    # (0,1) (core-variant: core 0's halo half fully masked), maddr covers the
    # generic window pair.
    madd0_d = nc.dram_tensor("madd0", [128, 512], BF16, kind="ExternalInput")
    maddr_d = nc.dram_tensor("maddr", [128, 512], BF16, kind="ExternalInput")
    out_d = nc.dram_tensor("out", [TLOC, C], F32, kind="ExternalOutput")

    with tile.TileContext(nc) as tc:
        with (
            tc.tile_pool(name="const", bufs=1) as constp,
            tc.tile_pool(name="xt", bufs=KT) as xtp,
            tc.tile_pool(name="w", bufs=16) as wp,
            tc.tile_pool(name="qt", bufs=KT) as qtp,
            tc.tile_pool(name="kt", bufs=KT) as ktp,
            tc.tile_pool(name="v", bufs=NKB) as vp,
            tc.tile_pool(name="yt", bufs=KT) as ytp,
            tc.tile_pool(name="att", bufs=6) as attp,
            tc.tile_pool(name="pt", bufs=16) as ptp,
            tc.tile_pool(name="stat", bufs=4) as statp,
            tc.tile_pool(name="z", bufs=4) as zp,
            tc.tile_pool(name="psum", bufs=1, space="PSUM") as psp,
        ):
            # HAM warm-up: junk matmuls that run while the first DMAs land,
            # flipping the PE clock gate to 8/8 before real work begins
            junk = constp.tile([128, 512], BF16, name="junk")
            nc.vector.memset(junk[:], 0.0)
            ps_w = psp.tile([128, 512], F32, tag="y", bufs=2, name="warm")
            for _ in range(4):
                nc.tensor.matmul(ps_w[:], junk[:, 0:128], junk[:], start=True,
                                 stop=True)
            ident = constp.tile([128, 128], BF16, name="ident")
            make_identity(nc, ident[:])

            madd0 = constp.tile([128, 512], BF16, name="madd0")
            maddr = constp.tile([128, 512], BF16, name="maddr")
            nc.sync.dma_start(madd0[:], madd0_d[:])
            nc.sync.dma_start(maddr[:], maddr_d[:])

            def load_w(dram, base, k):
                w = wp.tile([128, C], BF16, name=f"{base}{k}", tag="w", bufs=16)
                nc.sync.dma_start(w[:], dram[k * 128:(k + 1) * 128, :])
                return w

            # interleave x^T(fp8+bf16) and Wq tile loads so the first
            # q-projection accumulation chain starts as early as possible
            xt_t, xt8_t, wq8_t = [], [], []
            for a in range(4):
                x8 = xtp.tile([128, 2, KV], FP8, name=f"xt8_{a}", tag="xt8",
                              bufs=4)
                nc.sync.dma_start(
                    x8[:].rearrange("p a b -> p (a b)"),
                    xt8_d[a * 128:(a + 1) * 128, :])
                xt8_t.append(x8)
                w8 = wp.tile([128, 2, C], FP8, name=f"wq8_{a}", tag="w8",
                             bufs=8)
                nc.sync.dma_start(
                    w8[:].rearrange("p a b -> p (a b)"),
                    wqt8_d[a * 128:(a + 1) * 128, :])
                wq8_t.append(w8)
            for a in range(KT):
                xt = xtp.tile([128, KV], BF16, name=f"xt{a}", tag="xt", bufs=KT)
                nc.sync.dma_start(xt[:], xt_d[a * 128:(a + 1) * 128, :])
                xt_t.append(xt)
            wk8_t = []
            for a in range(4):
                w8 = wp.tile([128, 2, C], FP8, name=f"wk8_{a}", tag="w8",
                             bufs=8)
                nc.sync.dma_start(
                    w8[:].rearrange("p a b -> p (a b)"),
                    wkt8_d[a * 128:(a + 1) * 128, :])
                wk8_t.append(w8)

            DR = mybir.MatmulPerfMode.DoubleRow

            # ---- q^T projection (fp8 DoubleRow): out (o, t) tiles [128, 512]
            qt_t = []
            for o in range(KT):
                ps = psp.tile([128, 512], F32, tag="proj", bufs=3, name=f"psq{o}")
                for k in range(4):
                    nc.tensor.matmul(
                        ps[:],
                        wq8_t[k][:, :, o * 128:(o + 1) * 128],
                        xt8_t[k][:, :, HALO:],
                        start=(k == 0),
                        stop=(k == 3),
                        perf_mode=DR,
                    )
                qt = qtp.tile([128, TLOC], BF16, name=f"qt{o}", tag="qt", bufs=KT)
                nc.vector.tensor_copy(qt[:], ps[:])
                qt_t.append(qt)

            # ---- k^T projection (fp8 DoubleRow): out (o, t) tiles [128, 640]
            kt_t = []
            for o in range(KT):
                kt = ktp.tile([128, KV], BF16, name=f"kt{o}", tag="kt", bufs=KT)
                for n0, nw in ((0, 384), (384, 256)):
                    ps = psp.tile([128, 512], F32, tag="proj", bufs=3,
                                  name=f"psk{o}_{n0}")
                    for k in range(4):
                        nc.tensor.matmul(
                            ps[:, :nw],
                            wk8_t[k][:, :, o * 128:(o + 1) * 128],
                            xt8_t[k][:, :, n0:n0 + nw],
                            start=(k == 0),
                            stop=(k == 3),
                            perf_mode=DR,
                        )
                    nc.vector.tensor_copy(kt[:, n0:n0 + nw], ps[:, :nw])
                kt_t.append(kt)

            wv_t = [load_w(wvt_d, "wv", k) for k in range(KT)]
            wo_t = [load_w(wot_d, "wo", k) for k in range(KT)]

            # ---- attention front for head-pair g: scores + softmax + P^T.
            # Only needs qt[g]/kt[g]; interleaved with the V projection.
            pt_all = [None] * KT

            p_all = [None] * KT

            def attn_front(g):
                den = statp.tile([128, 8], F32, tag="den", bufs=4,
                                 name=f"den{g}")
                rec = statp.tile([128, 8], F32, tag="rec", bufs=4,
                                 name=f"rec{g}")
                e_t = {}
                for h in (0, 1):
                    ho = h * 64
                    e = attp.tile([128, 4 * 256], BF16, tag="e", bufs=6,
                                  name=f"e{g}_{h}")
                    e_t[h] = e
                    for qp in range(2):
                        madd = madd0 if qp == 0 else maddr
                        ps_s = psp.tile([128, 512], F32, tag="s", bufs=3,
                                        name=f"s{g}_{h}_{qp}")
                        for i in range(2):
                            qb = 2 * qp + i
                            # PE preloads the additive mask into PSUM, then
                            # the scores accumulate on top
                            nc.tensor.matmul(
                                ps_s[:, i * 256:(i + 1) * 256],
                                ident[:], madd[:, i * 256:(i + 1) * 256],
                                start=True, stop=False,
                                skip_group_check=True)
                            nc.tensor.matmul(
                                ps_s[:, i * 256:(i + 1) * 256],
                                qt_t[g][ho:ho + 64, qb * 128:(qb + 1) * 128],
                                kt_t[g][ho:ho + 64, qb * 128:qb * 128 + 256],
                                start=False,
                                stop=True,
                                skip_group_check=True,
                            )
                        # exp straight out of PSUM with per-window row sums
                        for i in range(2):
                            qb = 2 * qp + i
                            nc.scalar.activation(
                                e[:, qb * 256:(qb + 1) * 256],
                                ps_s[:, i * 256:(i + 1) * 256], EXP,
                                scale=1.0 / 512.0,
                                accum_out=den[:, h * 4 + qb:h * 4 + qb + 1])
                nc.vector.reciprocal(rec[:], den[:])
                for h in (0, 1):
                    p = attp.tile([128, 4 * 256], BF16, tag="p", bufs=6,
                                  name=f"p{g}_{h}")
                    p_all[g] = p_all[g] or {}
                    p_all[g][h] = p
                    if NORM_MODE == "gpsimd_bcast":
                        nc.gpsimd.tensor_tensor(
                            out=p[:].rearrange("p (b w) -> p b w", b=4),
                            in0=e_t[h][:].rearrange("p (b w) -> p b w", b=4),
                            in1=rec[:, h * 4:(h + 1) * 4].unsqueeze(-1)
                                .broadcast_to([128, 4, 256]),
                            op=MULT,
                        )
                    else:
                        for qb in range(NQB):
                            nc.vector.tensor_scalar_mul(
                                p[:, qb * 256:(qb + 1) * 256],
                                e_t[h][:, qb * 256:(qb + 1) * 256],
                                rec[:, h * 4 + qb:h * 4 + qb + 1])
            def attn_tposes(g):
                # P^T via PE transposes; pt window layout: kv block jb's
                # 256 q-cols are [128(jb-1), 128(jb+1)).
                for h in (0, 1):
                    p = p_all[g][h]
                    pt = ptp.tile([128, 256 * NKB], BF16, tag="pt", bufs=16,
                                  name=f"pt{g}_{h}")
                    for qp in range(2):
                        ps_t = psp.tile([128, 512], BF16, tag="s", bufs=3,
                                        name=f"t{g}_{h}_{qp}")
                        for i in range(2):
                            qb = 2 * qp + i
                            nc.tensor.transpose(
                                ps_t[:, i * 256:i * 256 + 128],
                                p[:, qb * 256:qb * 256 + 128], ident[:])
                            nc.tensor.transpose(
                                ps_t[:, i * 256 + 128:i * 256 + 256],
                                p[:, qb * 256 + 128:qb * 256 + 256],
                                ident[:])
                        for i in range(2):
                            qb = 2 * qp + i
                            nc.vector.tensor_copy(
                                pt[:, qb * 256 + 128:qb * 256 + 256],
                                ps_t[:, i * 256:i * 256 + 128])
                            nc.vector.tensor_copy(
                                pt[:, (qb + 1) * 256:(qb + 1) * 256 + 128],
                                ps_t[:, i * 256 + 128:i * 256 + 256])
                    pt_all[g] = pt_all[g] or {}
                    pt_all[g][h] = pt

            def v_chain(tb):
                v = vp.tile([128, C], BF16, name=f"v{tb}", tag="v", bufs=NKB)
                for n0 in (0, 512):
                    ps = psp.tile([128, 512], F32, tag="proj", bufs=3,
                                  name=f"psv{tb}_{n0}")
                    for k in range(KT):
                        nc.tensor.matmul(
                            ps[:],
                            xt_t[k][:, tb * 128:(tb + 1) * 128],
                            wv_t[k][:, n0:n0 + 512],
                            start=(k == 0),
                            stop=(k == KT - 1),
                        )
                    nc.vector.tensor_copy(v[:, n0:n0 + 512], ps[:])
                return v


            # ---- PV per head pair
            yt_t = [None] * KT

            def attn_pv(g):
                # start-flag-split accumulation: each q-block region of one
                # PSUM tile is its own 2-instruction group (kv blocks jb=b
                # then jb=b+1), so a single bank holds the full y and the
                # next group pipelines into the other buffer
                y = psp.tile([128, TLOC], F32, tag="y", bufs=2, name=f"y{g}")
                for jb in range(NKB):
                    for h in (0, 1):
                        ho = h * 64
                        for b in (jb - 1, jb):
                            if not (0 <= b < NQB):
                                continue
                            pc = jb * 256 + (0 if b == jb - 1 else 128)
                            nc.tensor.matmul(
                                y[ho:ho + 64, b * 128:(b + 1) * 128],
                                v_t[jb][:, (2 * g + h) * 64:
                                        (2 * g + h + 1) * 64],
                                pt_all[g][h][:, pc:pc + 128],
                                start=(jb == b),
                                stop=(jb == b + 1),
                                tile_position=(0, ho) if ho else None,
                                skip_group_check=True,
                            )
                yt = ytp.tile([128, TLOC], BF16, name=f"yt{g}", tag="yt",
                              bufs=KT)
                nc.scalar.copy(yt[:], y[:])
                yt_t[g] = yt

            # interleave: softmax fronts run one group ahead of their
            # transposes so the PE never blocks on the DVE normalize; V
            # chains and PV groups weave between
            v_t = []
            attn_front(0)
            v_t.append(v_chain(0))
            for g in range(1, NKB):
                attn_front(g)
                attn_tposes(g - 1)
                v_t.append(v_chain(g))
            attn_front(5)
            attn_tposes(4)
            attn_pv(0)
            attn_front(6)
            attn_tposes(5)
            attn_pv(1)
            attn_front(7)
            attn_tposes(6)
            attn_pv(2)
            attn_tposes(7)
            attn_pv(3)
            attn_pv(4)
            attn_pv(5)

            # ---- output projection z = y @ Wo^T, split into halves so the
            # first half overlaps the attention tail
            za_t = []

            def z_half_a():
                for tb in range(NQB):
                    for n0 in (0, 512):
                        ps = psp.tile([128, 512], F32, tag="proj", bufs=3,
                                      name=f"psza{tb}_{n0}")
                        for o in range(6):
                            nc.tensor.matmul(
                                ps[:],
                                yt_t[o][:, tb * 128:(tb + 1) * 128],
                                wo_t[o][:, n0:n0 + 512],
                                start=(o == 0),
                                stop=(o == 5),
                            )
                        za = zp.tile([128, 512], F32, name=f"za{tb}_{n0}",
                                     tag="za", bufs=8)
                        nc.vector.tensor_copy(za[:], ps[:])
                        za_t.append(za)

            z_half_a()
            for g in range(6, KT):
                attn_pv(g)

            for tb in range(NQB):
                zt = zp.tile([128, C], F32, name=f"z{tb}", tag="z", bufs=2)
                for ni, n0 in enumerate((0, 512)):
                    ps = psp.tile([128, 512], F32, tag="proj", bufs=3,
                                  name=f"pszb{tb}_{n0}")
                    for o in range(6, KT):
                        nc.tensor.matmul(
                            ps[:],
                            yt_t[o][:, tb * 128:(tb + 1) * 128],
                            wo_t[o][:, n0:n0 + 512],
                            start=(o == 6),
                            stop=(o == KT - 1),
                        )
                    nc.vector.tensor_tensor(out=zt[:, n0:n0 + 512], in0=ps[:],
                                            in1=za_t[tb * 2 + ni][:], op=ADD)
                eng = nc.sync if tb % 2 == 0 else nc.scalar
                eng.dma_start(out_d[tb * 128:(tb + 1) * 128, :], zt[:])

    nc.compile()
    return nc


def _masks():
    # additive masks for a [q 128, 512] score pair-tile; each 256-wide
    # window: cols 0:128 = prev kv block (valid iff k > r, strictly upper),
    # cols 128:256 = diag kv block (valid iff k-128 <= r).
    r = np.arange(128)[:, None]
    k = np.arange(256)[None, :]
    prev = (k < 128) & (k > r)
    diag = (k >= 128) & ((k - 128) <= r)
    win = np.where(prev | diag, 0.0, -1e9).astype(np.float32)
    first0 = np.where(diag & ~prev, 0.0, -1e9).astype(np.float32)
    maddr = np.concatenate([win, win], axis=1)
    madd0 = np.concatenate([win, win], axis=1)
    madd0_c0 = np.concatenate([first0, win], axis=1)
    return (madd0.astype(ml_dtypes.bfloat16),
            madd0_c0.astype(ml_dtypes.bfloat16),
            maddr.astype(ml_dtypes.bfloat16))


F8NP = ml_dtypes.float8_e4m3


def _dr_pack(a):
    # [C, cols] -> DoubleRow pairs [C//2, 2*cols]: row (2k+i)*128+p of a
    # becomes dram[k*128+p, i*cols:(i+1)*cols]
    cols = a.shape[1]
    b = a.reshape(4, 2, 128, cols).transpose(0, 2, 1, 3)
    return np.ascontiguousarray(b.reshape(C // 2, 2 * cols))


def make_in_maps(x, Wq, Wk, Wv, Wo):
    x = np.asarray(x, dtype=np.float32)
    xt = np.ascontiguousarray(x.reshape(T, C).T.astype(ml_dtypes.bfloat16))
    wkt = np.ascontiguousarray(np.asarray(Wk, np.float32).T.astype(ml_dtypes.bfloat16))
    wvt = np.ascontiguousarray(np.asarray(Wv, np.float32).T.astype(ml_dtypes.bfloat16))
    wot = np.ascontiguousarray(np.asarray(Wo, np.float32).T.astype(ml_dtypes.bfloat16))
    madd0, madd0_c0, maddr = _masks()
    # fp8 weights pre-scaled by 8 to stay clear of e4m3 subnormals; the
    # (8*8*sqrt(DH))=512 score descale folds into the exp activation scale
    wqt8 = _dr_pack((np.asarray(Wq, np.float32).T * 8.0).astype(F8NP))
    wkt8 = _dr_pack((np.asarray(Wk, np.float32).T * 8.0).astype(F8NP))

    in_maps = []
    for c in range(NCORES):
        t0 = c * TLOC
        xs = np.zeros((C, KV), dtype=ml_dtypes.bfloat16)
        lo = t0 - HALO
        src_lo = max(lo, 0)
        xs[:, src_lo - lo:] = xt[:, src_lo:t0 + TLOC]
        in_maps.append(
            {
                "xt": xs,
                "xt8": _dr_pack(xs.astype(np.float32).astype(F8NP)),
                "wqt8": wqt8,
                "wkt8": wkt8,
                "wvt": wvt,
                "wot": wot,
                "madd0": madd0_c0 if c == 0 else madd0,
                "maddr": maddr,
            }
        )
    return in_maps


def get_nc():
    if "nc" not in _cached:
        _cached["nc"] = build_nc()
    return _cached["nc"]


def kernel(x, Wq, Wk, Wv, Wo):
    in_maps = make_in_maps(x, Wq, Wk, Wv, Wo)
    res = run_bass_kernel_spmd(get_nc(), in_maps, list(range(NCORES)))
    out = np.concatenate([res.results[c]["out"] for c in range(NCORES)], axis=0)
    return out.reshape(1, T, C)


if __name__ == "__main__":
    rng = np.random.default_rng(0)
    ins = {
        "x": rng.standard_normal((1, T, C), dtype=np.float32),
        "Wq": rng.standard_normal((C, C), dtype=np.float32) * 0.02,
        "Wk": rng.standard_normal((C, C), dtype=np.float32) * 0.02,
        "Wv": rng.standard_normal((C, C), dtype=np.float32) * 0.02,
        "Wo": rng.standard_normal((C, C), dtype=np.float32) * 0.02,
    }
    out = kernel(**ins)
    print(out.shape, out.dtype, np.abs(out).mean())
